# revision 9
# baseline (speedup 1.0000x reference)
"""Trainium2 Bass kernel for Bahdanau-style attention.

reference:
    energy = tanh(enc @ W_enc + (dec @ W_dec + b_att)[:, None, :])   # (B,S,D)
    attn   = softmax(energy @ v, axis=S)                              # (B,S)
    out    = (attn[:, :, None] * enc).sum(S)[:, None, :]              # (B,1,E2)

Sharding: data-parallel over batch, 4 batches per core on 8 cores.

Per-core program (B'=4, S=2048, E2=1024, D=512), all fp32 in HBM:
  pass A: stream enc in a transposed-packed layout [e%128, (t, e//128)]
          (the HBM read order is still row-sequential), PE-matmul with
          W_enc chunks (float32r = full-rate fp32 path), tanh(+bias) on
          ScalarE -> energies [d, t] bf16, dot with v on PE -> logits
          [t,1] per 128-block, exp on ScalarE -> weights w.
          Softmax is computed WITHOUT max subtraction: |logit| <=
          ||v||_1 ~ 9, exp() is safe in fp32.
  pass B: re-stream enc in natural layout [t%128, e], PE-matmul with w
          as stationary -> U[e] = sum_t w_t enc[t,e] and Z = sum_t w_t
          accumulated in PSUM; out = U * (1/Z).
"""

import numpy as np

B, S, E2, D = 32, 2048, 1024, 512
NCORES = 8
BPC = B // NCORES          # batches per core
T = 512                    # s-tile size
NST = S // T               # s-tiles per batch
EC = E2 // 128             # e2 chunks (8)
NDB = D // 128             # d blocks (4)
TBLK = T // 128            # 128-blocks per s-tile (4)

_CACHE = {}
import os as _os
TKIND = _os.environ.get("TKIND", "chunked")  # transposed-load style: chunked|packed


def _build_nc():
    import concourse.bass as bass
    import concourse.tile as tile
    from concourse import bacc, mybir

    f32 = mybir.dt.float32
    f32r = mybir.dt.float32r
    bf16 = mybir.dt.bfloat16
    AF = mybir.ActivationFunctionType

    nc = bacc.Bacc(None, target_bir_lowering=False, debug=False)

    enc = nc.declare_dram_parameter("enc", [BPC, S, E2], f32r, isOutput=False)
    lhd = nc.declare_dram_parameter("lhd", [BPC, D], f32r, isOutput=False)
    w_att = nc.declare_dram_parameter("w_att", [E2 + D, D], f32r, isOutput=False)
    b_att = nc.declare_dram_parameter("b_att", [D], f32, isOutput=False)
    v = nc.declare_dram_parameter("v", [D], f32, isOutput=False)
    out = nc.declare_dram_parameter("out", [BPC, 1, E2], f32, isOutput=True)

    with tile.TileContext(nc) as tc:
        with (
            tc.tile_pool(name="weights", bufs=1) as wpool,
            tc.tile_pool(name="consts", bufs=1) as cpool,
            tc.tile_pool(name="enct", bufs=3) as tpool,
            tc.tile_pool(name="encnat", bufs=3) as npool,
            tc.tile_pool(name="energies", bufs=8) as epool,
            tc.tile_pool(name="small", bufs=2) as spool,
            tc.tile_pool(name="psume", bufs=3, space=bass.MemorySpace.PSUM) as psume,
            tc.tile_pool(name="psuml", bufs=1, space=bass.MemorySpace.PSUM) as psuml,
            tc.tile_pool(name="psumu", bufs=2, space=bass.MemorySpace.PSUM) as psumu,
        ):
            # ---- setup: weights + per-batch bias = dec@W_dec + b_att ----
            wenc = wpool.tile([128, EC, D], f32r)  # [p, c, d]; W_enc[c*128+p, d]
            nc.sync.dma_start(
                wenc[:], w_att[:E2, :].rearrange("(c p) d -> p c d", p=128)
            )
            wdec = wpool.tile([128, NDB, NDB, 128], f32r)  # [p, ki, mo, m]
            nc.sync.dma_start(
                wdec[:],
                w_att[E2:, :].rearrange("(ki p) (mo m) -> p ki mo m", p=128, m=128),
            )
            battT = cpool.tile([128, NDB], f32)  # [p, ki] = b_att[ki*128+p]
            nc.sync.dma_start(battT[:], b_att.rearrange("(ki p) -> p ki", p=128))
            vT = cpool.tile([128, NDB], f32)
            nc.sync.dma_start(vT[:], v.rearrange("(ki p) -> p ki", p=128))
            vb = cpool.tile([128, NDB], bf16)
            nc.vector.tensor_copy(vb[:], vT[:])
            lhdT = cpool.tile([128, NDB, BPC], f32r)  # [p, ki, b]
            lhd_r = lhd.rearrange("b (ki p) -> p ki b", p=128)
            for ki in range(NDB):
                nc.sync.dma_start(lhdT[:, ki, :], lhd_r[:, ki, :])

            bias = cpool.tile([128, NDB, BPC], f32)  # [p, mo, b]
            for mo in range(NDB):
                psdp = psume.tile([128, BPC], f32, tag="pse")
                for ki in range(NDB):
                    nc.tensor.matmul(
                        psdp[:],
                        wdec[:, ki, mo, :],
                        lhdT[:, ki, :],
                        start=(ki == 0),
                        stop=(ki == NDB - 1),
                    )
                nc.vector.tensor_scalar_add(
                    bias[:, mo, :], psdp[:], battT[:, mo : mo + 1]
                )

            # ---- main loop over this core's batches ----
            for b in range(BPC):
                w_all = spool.tile([128, NST * TBLK], f32r)  # [p, st*4+tb]
                zall = spool.tile([128, NST], f32)  # per-partition exp sums

                # pass A: energies + logits + exp
                for st in range(NST):
                    hbm_t = enc[b, st * T : (st + 1) * T, :].rearrange(
                        "t (c p) -> p t c", p=128
                    )
                    if TKIND == "packed":
                        # one fully-sequential HBM read; rhs slices stride-8
                        encT = tpool.tile([128, T, EC], f32r)  # [p, t, c]
                        nc.sync.dma_start(encT[:], hbm_t)
                        rhs_of = lambda c, e=encT: e[:, :, c]
                    else:
                        # 8 per-chunk DMAs (512B@4KiB stride); unit-stride rhs
                        encT = tpool.tile([128, EC, T], f32r)  # [p, c, t]
                        for c in range(EC):
                            nc.sync.dma_start(encT[:, c, :], hbm_t[:, :, c])
                        rhs_of = lambda c, e=encT: e[:, c, :]
                    engs = []
                    for db in range(NDB):
                        pse = psume.tile([128, T], f32, tag="pse")
                        for c in range(EC):
                            nc.tensor.matmul(
                                pse[:],
                                wenc[:, c, db * 128 : (db + 1) * 128],
                                rhs_of(c),
                                start=(c == 0),
                                stop=(c == EC - 1),
                            )
                        eng = epool.tile([128, T], bf16, tag="eng")
                        nc.scalar.activation(
                            eng[:], pse[:], AF.Tanh, bias=bias[:, db, b : b + 1]
                        )
                        engs.append(eng)
                    psl = psuml.tile([128, TBLK], f32)
                    for tb in range(TBLK):
                        for db in range(NDB):
                            nc.tensor.matmul(
                                psl[:, tb : tb + 1],
                                engs[db][:, tb * 128 : (tb + 1) * 128],
                                vb[:, db : db + 1],
                                start=(db == 0),
                                stop=(db == NDB - 1),
                            )
                    nc.scalar.activation(
                        w_all[:, st * TBLK : (st + 1) * TBLK],
                        psl[:],
                        AF.Exp,
                        accum_out=zall[:, st : st + 1],
                    )

                # pass B: U = sum_t w_t * enc[t, :]
                psu0 = psumu.tile([1, 512], f32, tag="psu")
                psu1 = psumu.tile([1, 512], f32, tag="psu")
                ncols = NST * TBLK
                for st in range(NST):
                    nat = npool.tile([128, TBLK, E2], f32r)  # [p, tb, e]
                    nc.sync.dma_start(
                        nat[:],
                        enc[b, st * T : (st + 1) * T, :].rearrange(
                            "(tb p) e -> p tb e", p=128
                        ),
                    )
                    for tb in range(TBLK):
                        col = st * TBLK + tb
                        first, last = col == 0, col == ncols - 1
                        wcol = w_all[:, col : col + 1]
                        nc.tensor.matmul(
                            psu0[:], wcol, nat[:, tb, 0:512],
                            start=first, stop=last,
                        )
                        nc.tensor.matmul(
                            psu1[:], wcol, nat[:, tb, 512:1024],
                            start=first, stop=last,
                        )

                # Z = sum of all weights: DVE free-reduce + GpSimd partition-reduce
                zred = spool.tile([128, 1], f32)
                nc.vector.tensor_reduce(
                    zred[:], zall[:], mybir.AxisListType.X, mybir.AluOpType.add
                )
                zfin = spool.tile([1, 1], f32)
                nc.gpsimd.tensor_reduce(
                    zfin[:], zred[:], mybir.AxisListType.C, mybir.AluOpType.add
                )
                recip = spool.tile([1, 1], f32)
                nc.vector.reciprocal(recip[:], zfin[:])
                outsb = spool.tile([1, E2], f32)
                nc.scalar.activation(
                    outsb[:, 0:512], psu0[:], AF.Copy, scale=recip[:]
                )
                nc.scalar.activation(
                    outsb[:, 512:1024], psu1[:], AF.Copy, scale=recip[:]
                )
                nc.sync.dma_start(out[b], outsb[:])

    nc.compile()
    return nc


def _get_nc():
    if "nc" not in _CACHE:
        _CACHE["nc"] = _build_nc()
    return _CACHE["nc"]


def kernel(output_encoder, last_hidden_decoder, W_att, b_att, v):
    from concourse.bass_utils import run_bass_kernel_spmd

    nc = _get_nc()
    output_encoder = np.ascontiguousarray(output_encoder, dtype=np.float32)
    last_hidden_decoder = np.ascontiguousarray(last_hidden_decoder, dtype=np.float32)
    W_att = np.ascontiguousarray(W_att, dtype=np.float32)
    b_att = np.ascontiguousarray(b_att, dtype=np.float32)
    v = np.ascontiguousarray(v, dtype=np.float32)

    in_maps = []
    for c in range(NCORES):
        sl = slice(c * BPC, (c + 1) * BPC)
        in_maps.append(
            {
                "enc": output_encoder[sl],
                "lhd": last_hidden_decoder[sl],
                "w_att": W_att,
                "b_att": b_att,
                "v": v,
            }
        )
    res = run_bass_kernel_spmd(nc, in_maps, list(range(NCORES)))
    return np.concatenate([res.results[c]["out"] for c in range(NCORES)], axis=0)


# revision 11
# speedup vs baseline: 74.5396x; 74.5396x over previous
"""Trainium2 Bass kernel for Bahdanau-style attention.

reference:
    energy = tanh(enc @ W_enc + (dec @ W_dec + b_att)[:, None, :])   # (B,S,D)
    attn   = softmax(energy @ v, axis=S)                              # (B,S)
    out    = (attn[:, :, None] * enc).sum(S)[:, None, :]              # (B,1,E2)

Sharding: data-parallel over batch, 4 batches per core on 8 cores.

Per-core program (B'=4, S=2048, E2=1024, D=512), all fp32 in HBM:
  pass A: stream enc in a transposed-packed layout [e%128, (t, e//128)]
          (the HBM read order is still row-sequential), PE-matmul with
          W_enc chunks (float32r = full-rate fp32 path), tanh(+bias) on
          ScalarE -> energies [d, t] bf16, dot with v on PE -> logits
          [t,1] per 128-block, exp on ScalarE -> weights w.
          Softmax is computed WITHOUT max subtraction: |logit| <=
          ||v||_1 ~ 9, exp() is safe in fp32.
  pass B: re-stream enc in natural layout [t%128, e], PE-matmul with w
          as stationary -> U[e] = sum_t w_t enc[t,e] and Z = sum_t w_t
          accumulated in PSUM; out = U * (1/Z).
"""

import numpy as np

B, S, E2, D = 32, 2048, 1024, 512
NCORES = 8
BPC = B // NCORES          # batches per core
T = 512                    # s-tile size
NST = S // T               # s-tiles per batch
EC = E2 // 128             # e2 chunks (8)
NDB = D // 128             # d blocks (4)
TBLK = T // 128            # 128-blocks per s-tile (4)

_CACHE = {}
import os as _os
TKIND = _os.environ.get("TKIND", "chunked")  # transposed-load style: chunked|packed


def _build_nc():
    import concourse.bass as bass
    import concourse.tile as tile
    from concourse import bacc, bass_isa, mybir

    f32 = mybir.dt.float32
    f32r = mybir.dt.float32r
    bf16 = mybir.dt.bfloat16
    AF = mybir.ActivationFunctionType

    nc = bacc.Bacc(None, target_bir_lowering=False, debug=False)

    enc = nc.declare_dram_parameter("enc", [BPC, S, E2], f32r, isOutput=False)
    lhd = nc.declare_dram_parameter("lhd", [BPC, D], f32r, isOutput=False)
    w_att = nc.declare_dram_parameter("w_att", [E2 + D, D], f32r, isOutput=False)
    b_att = nc.declare_dram_parameter("b_att", [D], f32, isOutput=False)
    v = nc.declare_dram_parameter("v", [D], f32, isOutput=False)
    out = nc.declare_dram_parameter("out", [BPC, 1, E2], f32, isOutput=True)

    with tile.TileContext(nc) as tc:
        with (
            tc.tile_pool(name="weights", bufs=1) as wpool,
            tc.tile_pool(name="consts", bufs=1) as cpool,
            tc.tile_pool(name="enct", bufs=3) as tpool,
            tc.tile_pool(name="encnat", bufs=3) as npool,
            tc.tile_pool(name="energies", bufs=8) as epool,
            tc.tile_pool(name="small", bufs=2) as spool,
            tc.tile_pool(name="psume", bufs=3, space=bass.MemorySpace.PSUM) as psume,
            tc.tile_pool(name="psuml", bufs=1, space=bass.MemorySpace.PSUM) as psuml,
            tc.tile_pool(name="psumu", bufs=2, space=bass.MemorySpace.PSUM) as psumu,
        ):
            # ---- setup: weights + per-batch bias = dec@W_dec + b_att ----
            wenc = wpool.tile([128, EC, D], f32r)  # [p, c, d]; W_enc[c*128+p, d]
            nc.sync.dma_start(
                wenc[:], w_att[:E2, :].rearrange("(c p) d -> p c d", p=128)
            )
            wdec = wpool.tile([128, NDB, NDB, 128], f32r)  # [p, ki, mo, m]
            nc.sync.dma_start(
                wdec[:],
                w_att[E2:, :].rearrange("(ki p) (mo m) -> p ki mo m", p=128, m=128),
            )
            battT = cpool.tile([128, NDB], f32)  # [p, ki] = b_att[ki*128+p]
            nc.sync.dma_start(battT[:], b_att.rearrange("(ki p) -> p ki", p=128))
            vT = cpool.tile([128, NDB], f32)
            nc.sync.dma_start(vT[:], v.rearrange("(ki p) -> p ki", p=128))
            vb = cpool.tile([128, NDB], bf16)
            nc.vector.tensor_copy(vb[:], vT[:])
            lhdT = cpool.tile([128, NDB, BPC], f32r)  # [p, ki, b]
            lhd_r = lhd.rearrange("b (ki p) -> p ki b", p=128)
            for ki in range(NDB):
                nc.sync.dma_start(lhdT[:, ki, :], lhd_r[:, ki, :])

            bias = cpool.tile([128, NDB, BPC], f32)  # [p, mo, b]
            for mo in range(NDB):
                psdp = psume.tile([128, BPC], f32, tag="pse")
                for ki in range(NDB):
                    nc.tensor.matmul(
                        psdp[:],
                        wdec[:, ki, mo, :],
                        lhdT[:, ki, :],
                        start=(ki == 0),
                        stop=(ki == NDB - 1),
                    )
                nc.vector.tensor_scalar_add(
                    bias[:, mo, :], psdp[:], battT[:, mo : mo + 1]
                )

            # ---- main loop over this core's batches ----
            for b in range(BPC):
                w_all = spool.tile([128, NST * TBLK], f32r)  # [p, st*4+tb]
                zall = spool.tile([128, NST], f32)  # per-partition exp sums

                # pass A: energies + logits + exp
                for st in range(NST):
                    hbm_t = enc[b, st * T : (st + 1) * T, :].rearrange(
                        "t (c p) -> p t c", p=128
                    )
                    if TKIND == "packed":
                        # one fully-sequential HBM read; rhs slices stride-8
                        encT = tpool.tile([128, T, EC], f32r)  # [p, t, c]
                        nc.sync.dma_start(encT[:], hbm_t)
                        rhs_of = lambda c, e=encT: e[:, :, c]
                    else:
                        # 8 per-chunk DMAs (512B@4KiB stride); unit-stride rhs
                        encT = tpool.tile([128, EC, T], f32r)  # [p, c, t]
                        for c in range(EC):
                            nc.sync.dma_start(encT[:, c, :], hbm_t[:, :, c])
                        rhs_of = lambda c, e=encT: e[:, c, :]
                    engs = []
                    for db in range(NDB):
                        pse = psume.tile([128, T], f32, tag="pse")
                        for c in range(EC):
                            nc.tensor.matmul(
                                pse[:],
                                wenc[:, c, db * 128 : (db + 1) * 128],
                                rhs_of(c),
                                start=(c == 0),
                                stop=(c == EC - 1),
                            )
                        eng = epool.tile([128, T], bf16, tag="eng")
                        nc.scalar.activation(
                            eng[:], pse[:], AF.Tanh, bias=bias[:, db, b : b + 1]
                        )
                        engs.append(eng)
                    psl = psuml.tile([128, TBLK], f32)
                    for tb in range(TBLK):
                        for db in range(NDB):
                            nc.tensor.matmul(
                                psl[:, tb : tb + 1],
                                engs[db][:, tb * 128 : (tb + 1) * 128],
                                vb[:, db : db + 1],
                                start=(db == 0),
                                stop=(db == NDB - 1),
                            )
                    nc.scalar.activation(
                        w_all[:, st * TBLK : (st + 1) * TBLK],
                        psl[:],
                        AF.Exp,
                        accum_out=zall[:, st : st + 1],
                    )

                # pass B: U = sum_t w_t * enc[t, :]
                psu0 = psumu.tile([1, 512], f32, tag="psu")
                psu1 = psumu.tile([1, 512], f32, tag="psu")
                ncols = NST * TBLK
                for st in range(NST):
                    nat = npool.tile([128, TBLK, E2], f32r)  # [p, tb, e]
                    nc.sync.dma_start(
                        nat[:],
                        enc[b, st * T : (st + 1) * T, :].rearrange(
                            "(tb p) e -> p tb e", p=128
                        ),
                    )
                    for tb in range(TBLK):
                        col = st * TBLK + tb
                        first, last = col == 0, col == ncols - 1
                        wcol = w_all[:, col : col + 1]
                        nc.tensor.matmul(
                            psu0[:], wcol, nat[:, tb, 0:512],
                            start=first, stop=last,
                        )
                        nc.tensor.matmul(
                            psu1[:], wcol, nat[:, tb, 512:1024],
                            start=first, stop=last,
                        )

                # Z = sum of all weights: DVE free-reduce + GpSimd partition-reduce
                zred = spool.tile([128, 1], f32)
                nc.vector.tensor_reduce(
                    zred[:], zall[:], mybir.AxisListType.X, mybir.AluOpType.add
                )
                zfin = spool.tile([128, 1], f32)
                nc.gpsimd.partition_all_reduce(
                    zfin[:], zred[:], channels=128, reduce_op=bass_isa.ReduceOp.add
                )
                recip = spool.tile([1, 1], f32)
                nc.vector.reciprocal(recip[:], zfin[0:1, :])
                outsb = spool.tile([1, E2], f32)
                nc.scalar.activation(
                    outsb[:, 0:512], psu0[:], AF.Copy, scale=recip[:]
                )
                nc.scalar.activation(
                    outsb[:, 512:1024], psu1[:], AF.Copy, scale=recip[:]
                )
                nc.sync.dma_start(out[b], outsb[:])

    nc.compile()
    return nc


def _get_nc():
    if "nc" not in _CACHE:
        _CACHE["nc"] = _build_nc()
    return _CACHE["nc"]


def kernel(output_encoder, last_hidden_decoder, W_att, b_att, v):
    from concourse.bass_utils import run_bass_kernel_spmd

    nc = _get_nc()
    output_encoder = np.ascontiguousarray(output_encoder, dtype=np.float32)
    last_hidden_decoder = np.ascontiguousarray(last_hidden_decoder, dtype=np.float32)
    W_att = np.ascontiguousarray(W_att, dtype=np.float32)
    b_att = np.ascontiguousarray(b_att, dtype=np.float32)
    v = np.ascontiguousarray(v, dtype=np.float32)

    in_maps = []
    for c in range(NCORES):
        sl = slice(c * BPC, (c + 1) * BPC)
        in_maps.append(
            {
                "enc": output_encoder[sl],
                "lhd": last_hidden_decoder[sl],
                "w_att": W_att,
                "b_att": b_att,
                "v": v,
            }
        )
    res = run_bass_kernel_spmd(nc, in_maps, list(range(NCORES)))
    return np.concatenate([res.results[c]["out"] for c in range(NCORES)], axis=0)
